# revision 12
# baseline (speedup 1.0000x reference)
"""Distributed Bass kernel for nn_LingoAuraLLM (8 TRN2 NeuronCores).

Sharding: megatron-style TP + sequence-parallel.
- tokens padded to 1088 (2 batches x 544); core c owns tokens [136c, 136(c+1))
- heads: core c owns heads 2c, 2c+1 (q/k/v column-parallel, o row-parallel)
- FFN rows [704c, 704(c+1)) (gate/up column-, down row-parallel)
- vocab rows [4000c, 4000(c+1)) for lm_head; CE via sumexp ReduceScatter
- NF4 weights shipped as bf16 codebook values + scales; dequant (scale mul) on device
"""
import numpy as np
import ml_dtypes

import concourse.bass as bass
import concourse.mybir as mybir
import concourse.tile as tile
from concourse import bacc
from concourse.bass_utils import run_bass_kernel_spmd
from concourse.masks import make_identity

F32 = mybir.dt.float32
BF16 = mybir.dt.bfloat16
AX = mybir.AxisListType.X
MULT = mybir.AluOpType.mult
ADD = mybir.AluOpType.add
MAXOP = mybir.AluOpType.max
AF = mybir.ActivationFunctionType

NCORE = 8
B, T, H, NH, HD = 2, 512, 2048, 16, 128
FFN, V, L, R, BLK = 5632, 32000, 2, 32, 64
VD, AD, VT, AT = 1024, 768, 256, 512
LORA_SCALE = 2.0
S = T + 2            # 514 real seq
SB = 544             # padded per-batch seq (= 4*136, %32==0)
TOK = 2 * SB         # 1088
TLOC = TOK // NCORE  # 136
HLOC = NH // NCORE   # 2 heads
OQ = HLOC * HD       # 256 qkv out rows per core
FLOC = FFN // NCORE  # 704
VLOC = V // NCORE    # 4000
PT = 256             # padded projector tokens per core
PD = 1024            # padded projector input dim
P2D = 2048           # padded projector hidden (2*d)
INV_SQRT_HD = float(1.0 / np.sqrt(HD))

NF4 = np.array([-1.0, -0.6961928009986877, -0.5250730514526367, -0.39491748809814453,
                -0.28444138169288635, -0.18477343022823334, -0.09105003625154495, 0.0,
                0.07958029955625534, 0.16093020141124725, 0.2461123913526535,
                0.33791524171829224, 0.44070982933044434, 0.5626170039176941,
                0.7229568362236023, 1.0], dtype=np.float32)

# token-free-dim chunks, batch aligned: (offset, len)
TOKCH = [(0, 512), (512, 32), (544, 512), (1056, 32)]
# token M-tiles for o/down/lm_head outputs
MT_TOK = [(i * 128, 128) for i in range(8)] + [(1024, 64)]
# ffn row tiles
MT_FFN = [(i * 128, 128) for i in range(5)] + [(640, 64)]
# per-batch q/key tiles for attention
QT_ATT = [(0, 128), (128, 128), (256, 128), (384, 128), (512, 32)]


def _bf(x):
    return np.ascontiguousarray(x).astype(ml_dtypes.bfloat16)


def _f32(x):
    return np.ascontiguousarray(x, dtype=np.float32)


# ---------------------------------------------------------------------------
# device kernel builder
# ---------------------------------------------------------------------------

_CACHE = {}


def _declare_inputs(nc):
    d = {}

    def inp(name, shape, dt=BF16):
        d[name] = nc.declare_dram_parameter(name, list(shape), dt, isOutput=False)

    # projector
    inp("pt_inT", (PD, PT))
    inp("pw1", (PD, P2D))
    inp("pb1", (128, P2D // 128), F32)       # b1 striped [128, 16]
    inp("pw2", (P2D, H))
    inp("pb2", (1, H), F32)
    inp("png", (1, H), F32)
    inp("pnb", (1, H), F32)
    inp("pmean_sel", (PT, 4))
    # embeddings / residual init
    inp("te_loc", (TLOC, H), F32)
    inp("emb_selT", (4, TLOC))
    # norms
    inp("attn_w", (L, 1, H), F32)
    inp("mlp_w", (L, 1, H), F32)
    inp("fin_w", (1, H), F32)
    # rope / attention
    inp("cos_t", (64, SB), F32)
    inp("sin_t", (64, SB), F32)
    inp("bias", (B, SB, SB))
    # per-layer quantized weights (values bf16, scales f32)
    inp("wq_v", (L, H, OQ)); inp("wq_s", (L, H // BLK, OQ), F32)
    inp("wk_v", (L, H, OQ)); inp("wk_s", (L, H // BLK, OQ), F32)
    inp("wv_v", (L, H, OQ)); inp("wv_s", (L, H // BLK, OQ), F32)
    inp("wo_v", (L, OQ, H)); inp("wo_s", (L, OQ // BLK, H), F32)
    inp("wg_v", (L, H, FLOC)); inp("wg_s", (L, H // BLK, FLOC), F32)
    inp("wu_v", (L, H, FLOC)); inp("wu_s", (L, H // BLK, FLOC), F32)
    inp("wd_v", (L, FLOC, H)); inp("wd_s", (L, FLOC // BLK, H), F32)
    # lora (kxm layouts)
    inp("aq", (L, H, R)); inp("ak", (L, H, R)); inp("av", (L, H, R)); inp("ao", (L, OQ, R))
    inp("bq", (L, R, OQ)); inp("bk", (L, R, OQ)); inp("bv", (L, R, OQ)); inp("bo", (L, R, H))
    # lm head + loss
    inp("lmh", (H, VLOC))
    inp("tgt_rows", (TLOC, H), F32)
    inp("nll_w", (TLOC, 1), F32)
    inp("tc_rows", (2, H), F32)
    inp("tc_kxm", (H, 2))
    inp("dsel", (4, 2), F32)
    inp("i2", (2, 2))
    inp("temp", (1, 1), F32)
    return d


def build_nc(debug_outputs=False):
    nc = bacc.Bacc("TRN2", target_bir_lowering=False, debug=False, num_devices=NCORE)
    D = _declare_inputs(nc)
    out_loss = nc.declare_dram_parameter("loss", [1, 1], F32, isOutput=True)
    dbg = {}
    if debug_outputs:
        dbg["xs0"] = nc.declare_dram_parameter("d_xs0", [TLOC, H], F32, isOutput=True)
        dbg["xs1"] = nc.declare_dram_parameter("d_xs1", [TLOC, H], F32, isOutput=True)
        dbg["xs2"] = nc.declare_dram_parameter("d_xs2", [TLOC, H], F32, isOutput=True)
        dbg["emb"] = nc.declare_dram_parameter("d_emb", [4, H], F32, isOutput=True)
        dbg["scal"] = nc.declare_dram_parameter("d_scal", [8, 1], F32, isOutput=True)
    CORES = list(range(NCORE))

    with tile.TileContext(nc) as tc:
        _build_body(nc, tc, D, out_loss, dbg, CORES)
    nc.finalize()
    return nc


def _build_body(nc, tc, D, out_loss, dbg, CORES):
    from contextlib import ExitStack
    ctx = ExitStack()
    with ctx:
        _build_body_inner(ctx, nc, tc, D, out_loss, dbg, CORES)


def _build_body_inner(ctx, nc, tc, D, out_loss, dbg, CORES):
    const = ctx.enter_context(tc.tile_pool(name="const", bufs=1))
    wstream = ctx.enter_context(tc.tile_pool(name="wstream", bufs=2))
    wkeep = ctx.enter_context(tc.tile_pool(name="wkeep", bufs=1))
    big = ctx.enter_context(tc.tile_pool(name="big", bufs=1))
    act = ctx.enter_context(tc.tile_pool(name="act", bufs=2))
    scr = ctx.enter_context(tc.tile_pool(name="scr", bufs=2))
    scr1 = ctx.enter_context(tc.tile_pool(name="scr1", bufs=1))
    psA = ctx.enter_context(tc.tile_pool(name="psA", bufs=2, space="PSUM"))
    psB = ctx.enter_context(tc.tile_pool(name="psB", bufs=2, space="PSUM"))
    psLM = ctx.enter_context(tc.tile_pool(name="psLM", bufs=1, space="PSUM"))
    psO = ctx.enter_context(tc.tile_pool(name="psO", bufs=1, space="PSUM"))
    dram = ctx.enter_context(tc.tile_pool(name="dram", bufs=2, space="DRAM"))
    dram1 = ctx.enter_context(tc.tile_pool(name="dram1", bufs=1, space="DRAM"))

    def expand_row(pool, tag, src_row_ap, p=128, dt=F32, w=H):
        t = pool.tile([p, w], dt, tag=tag)
        eng = nc.gpsimd if src_row_ap.dtype != dt else nc.sync
        eng.dma_start(t[:], src_row_ap.to_broadcast((p, w)))
        return t

    ident = const.tile([128, 128], BF16)
    make_identity(nc, ident)
    eps_sb = const.tile([128, 1], F32)
    nc.any.memset(eps_sb[:], 1e-5)

    cos_sb = const.tile([64, SB], F32)
    nc.sync.dma_start(cos_sb[:], D["cos_t"][:])
    sin_sb = const.tile([64, SB], F32)
    nc.sync.dma_start(sin_sb[:], D["sin_t"][:])
    bias_sb = {}
    for b in range(B):
        for qi, (qo, ql) in enumerate(QT_ATT):
            t = const.tile([ql, SB], BF16, tag=f"bias_{b}_{qi}")
            nc.sync.dma_start(t[:], D["bias"][b, qo:qo + ql, :])
            bias_sb[(b, qi)] = t
    selT = const.tile([4, TLOC], BF16)
    nc.sync.dma_start(selT[:], D["emb_selT"][:])

    MT_LOC = [(0, 128), (128, TLOC - 128)]
    x_s = []
    for (mo, ml) in MT_LOC:
        xt = big.tile([ml, H], F32, tag=f"xs_{mo}")
        nc.sync.dma_start(xt[:], D["te_loc"][mo:mo + ml, :])
        x_s.append(xt)
    contrast = const.tile([1, 1], F32)

    # =====================================================================
    # PHASE 1+2: projector, emb AllReduce, x_s init, contrast  (own pool)
    # =====================================================================
    with tc.tile_pool(name="projp", bufs=1) as projp:
        ptin = []
        for k in range(PD // 128):
            t = projp.tile([128, PT], BF16, tag=f"ptin{k}")
            nc.sync.dma_start(t[:], D["pt_inT"][k * 128:(k + 1) * 128, :])
            ptin.append(t)
        pb1_sb = projp.tile([128, P2D // 128], F32)
        nc.sync.dma_start(pb1_sb[:], D["pb1"][:])

        h1T = []
        for m in range(P2D // 128):
            ps = psA.tile([128, PT], F32, tag="ps_main")
            for k in range(PD // 128):
                w = wstream.tile([128, 128], BF16, tag="pw1t")
                nc.sync.dma_start(w[:], D["pw1"][k * 128:(k + 1) * 128, m * 128:(m + 1) * 128])
                nc.tensor.matmul(ps[:], w[:], ptin[k][:], start=(k == 0), stop=(k == PD // 128 - 1))
            h = projp.tile([128, PT], BF16, tag=f"h1T{m}")
            nc.scalar.activation(h[:], ps[:], AF.Relu, bias=pb1_sb[:, m:m + 1])
            h1T.append(h)

        pb2_exp = expand_row(projp, "pb2e", D["pb2"][0:1, :], dt=BF16)
        png_exp = expand_row(projp, "pnge", D["png"][0:1, :], dt=BF16)
        pnb_exp = expand_row(projp, "pnbe", D["pnb"][0:1, :], dt=BF16)

        ln_bf = []
        for mt in range(PT // 128):
            h2 = projp.tile([128, H], F32, tag="h2")
            for n in range(H // 512):
                ps = psA.tile([128, 512], F32, tag="ps_main")
                for k in range(P2D // 128):
                    w = wstream.tile([128, 512], BF16, tag="pw2t")
                    nc.sync.dma_start(w[:], D["pw2"][k * 128:(k + 1) * 128, n * 512:(n + 1) * 512])
                    nc.tensor.matmul(ps[:], h1T[k][:, mt * 128:(mt + 1) * 128], w[:],
                                     start=(k == 0), stop=(k == P2D // 128 - 1))
                nc.vector.tensor_add(h2[:, n * 512:(n + 1) * 512], ps[:], pb2_exp[:, n * 512:(n + 1) * 512])
            mean = scr.tile([128, 1], F32, tag="m1")
            nc.vector.tensor_reduce(out=mean[:], in_=h2[:], op=ADD, axis=AX)
            nmean = scr.tile([128, 1], F32, tag="m2")
            nc.vector.tensor_scalar_mul(nmean[:], mean[:], -1.0 / H)
            nc.scalar.activation(h2[:], h2[:], AF.Identity, bias=nmean[:])
            fs = scr1.tile([128, H], F32, tag="fscr")
            ssq = scr.tile([128, 1], F32, tag="m3")
            nc.scalar.activation(fs[:], h2[:], AF.Square, accum_out=ssq[:])
            sd = scr.tile([128, 1], F32, tag="m4")
            nc.scalar.activation(sd[:], ssq[:], AF.Sqrt, scale=1.0 / H, bias=eps_sb[:])
            rstd = scr.tile([128, 1], F32, tag="m5")
            nc.vector.reciprocal(rstd[:], sd[:])
            nc.scalar.activation(h2[:], h2[:], AF.Copy, scale=rstd[:])
            nc.vector.tensor_mul(h2[:], h2[:], png_exp[:])
            nc.vector.tensor_add(h2[:], h2[:], pnb_exp[:])
            lb = projp.tile([128, H], BF16, tag=f"lnbf{mt}")
            nc.vector.tensor_copy(lb[:], h2[:])
            ln_bf.append(lb)

        psels = []
        for kt in range(PT // 128):
            pt_ = projp.tile([128, 4], BF16, tag=f"psel{kt}")
            nc.sync.dma_start(pt_[:], D["pmean_sel"][kt * 128:(kt + 1) * 128, :])
            psels.append(pt_)
        contrib = projp.tile([4, H], F32)
        for n in range(H // 512):
            ps = psA.tile([4, 512], F32, tag="ps_main")
            for kt in range(PT // 128):
                nc.tensor.matmul(ps[:], psels[kt][:], ln_bf[kt][:, n * 512:(n + 1) * 512],
                                 start=(kt == 0), stop=(kt == PT // 128 - 1))
            nc.vector.tensor_copy(contrib[:, n * 512:(n + 1) * 512], ps[:])
        cc_emb_in = dram1.tile([4, H], F32)
        cc_emb_out = dram1.tile([4, H], F32)
        nc.sync.dma_start(cc_emb_in[:], contrib[:])
        nc.gpsimd.collective_compute("AllReduce", ADD, replica_groups=[CORES],
                                     ins=[cc_emb_in.opt()], outs=[cc_emb_out.opt()])
        emb_rows = projp.tile([4, H], F32)
        nc.sync.dma_start(emb_rows[:], cc_emb_out[:])
        if dbg:
            nc.sync.dma_start(dbg["emb"][:], emb_rows[:])

        # x_s += emb_selT.T @ emb_rows
        emb_bf = projp.tile([4, H], BF16)
        nc.vector.tensor_copy(emb_bf[:], emb_rows[:])
        for ti, (mo, ml) in enumerate(MT_LOC):
            for n in range(H // 512):
                ps = psA.tile([ml, 512], F32, tag="ps_main")
                nc.tensor.matmul(ps[:], selT[:, mo:mo + ml], emb_bf[:, n * 512:(n + 1) * 512],
                                 start=True, stop=True)
                nc.vector.tensor_add(x_s[ti][:, n * 512:(n + 1) * 512],
                                     x_s[ti][:, n * 512:(n + 1) * 512], ps[:])
        if dbg:
            nc.sync.dma_start(dbg["xs0"][0:128, :], x_s[0][:])
            nc.sync.dma_start(dbg["xs0"][128:TLOC, :], x_s[1][:])

        # ---- contrastive loss (replicated) ----
        tc_sb = projp.tile([2, H], F32)
        nc.sync.dma_start(tc_sb[:], D["tc_rows"][:])
        sq4 = scr1.tile([4, H], F32, tag="fscr")
        n2v = scr.tile([4, 1], F32, tag="n2v")
        nc.scalar.activation(sq4[:], emb_rows[:], AF.Square, accum_out=n2v[:])
        sq2 = scr1.tile([2, H], F32, tag="fscr")
        n2t = scr.tile([2, 1], F32, tag="n2t")
        nc.scalar.activation(sq2[:], tc_sb[:], AF.Square, accum_out=n2t[:])

        def recip_norm(n2, p):
            nt = scr.tile([p, 1], F32, tag=f"nt{p}")
            nc.scalar.activation(nt[:], n2[:], AF.Sqrt)
            nc.vector.tensor_scalar_max(nt[:], nt[:], 1e-12)
            r = scr.tile([p, 1], F32, tag=f"rn{p}")
            nc.vector.reciprocal(r[:], nt[:])
            return r

        rv = recip_norm(n2v, 4)
        rtc = recip_norm(n2t, 2)
        ps_sim = psB.tile([4, 2], F32, tag="ps_small")
        for k in range(H // 128):
            tck = wstream.tile([128, 2], BF16, tag="tckxm")
            nc.sync.dma_start(tck[:], D["tc_kxm"][k * 128:(k + 1) * 128, :])
            ebT_ps = psB.tile([128, 4], BF16, tag="ps_small")
            nc.tensor.transpose(ebT_ps[:], emb_bf[:, k * 128:(k + 1) * 128], ident[0:4, 0:4])
            ebT = scr.tile([128, 4], BF16, tag="ebT")
            nc.vector.tensor_copy(ebT[:], ebT_ps[:])
            nc.tensor.matmul(ps_sim[:], ebT[:], tck[:], start=(k == 0), stop=(k == H // 128 - 1))
        sims = scr.tile([4, 2], F32, tag="sims")
        nc.scalar.activation(sims[:], ps_sim[:], AF.Copy, scale=rv[:])
        rtc_bf = scr.tile([2, 1], BF16, tag="rtcbf")
        nc.vector.tensor_copy(rtc_bf[:], rtc[:])
        i2sb = projp.tile([2, 2], BF16)
        nc.sync.dma_start(i2sb[:], D["i2"][:])
        ps_rt = psB.tile([1, 2], F32, tag="ps_small")
        nc.tensor.matmul(ps_rt[:], rtc_bf[:], i2sb[:], start=True, stop=True)
        rtc_row = scr.tile([1, 2], F32, tag="rtcrow")
        nc.vector.tensor_copy(rtc_row[:], ps_rt[:])
        rtc_b4 = scr.tile([4, 2], F32, tag="rtcb4")
        nc.gpsimd.partition_broadcast(rtc_b4[:], rtc_row[:], channels=4)
        nc.vector.tensor_mul(sims[:], sims[:], rtc_b4[:])
        tmp4 = scr.tile([4, 1], F32, tag="tmp4")
        nc.sync.dma_start(tmp4[:], D["temp"][0:1, :].to_broadcast((4, 1)))
        rtemp = scr.tile([4, 1], F32, tag="rtemp")
        nc.vector.reciprocal(rtemp[:], tmp4[:])
        nc.scalar.activation(sims[:], sims[:], AF.Copy, scale=rtemp[:])
        mx4 = scr.tile([4, 1], F32, tag="mx4")
        nc.vector.tensor_reduce(out=mx4[:], in_=sims[:], op=MAXOP, axis=AX)
        nmx4 = scr.tile([4, 1], F32, tag="nmx4")
        nc.vector.tensor_scalar_mul(nmx4[:], mx4[:], -1.0)
        e4 = scr.tile([4, 2], F32, tag="e4")
        s4 = scr.tile([4, 1], F32, tag="s4")
        nc.scalar.activation(e4[:], sims[:], AF.Exp, bias=nmx4[:], accum_out=s4[:])
        lse4 = scr.tile([4, 1], F32, tag="lse4")
        nc.scalar.activation(lse4[:], s4[:], AF.Ln)
        nc.vector.tensor_add(lse4[:], lse4[:], mx4[:])
        dsel_sb = projp.tile([4, 2], F32)
        nc.sync.dma_start(dsel_sb[:], D["dsel"][:])
        dmul = scr.tile([4, 2], F32, tag="dmul")
        nc.vector.tensor_mul(dmul[:], sims[:], dsel_sb[:])
        diag = scr.tile([4, 1], F32, tag="diag")
        nc.vector.tensor_reduce(out=diag[:], in_=dmul[:], op=ADD, axis=AX)
        term = scr.tile([4, 1], F32, tag="term")
        nc.vector.tensor_sub(term[:], diag[:], lse4[:])
        tsum = scr.tile([4, 1], F32, tag="tsum")
        nc.gpsimd.partition_all_reduce(tsum[:], term[:], channels=4,
                                       reduce_op=bass.bass_isa.ReduceOp.add)
        nc.vector.tensor_scalar_mul(contrast[:], tsum[0:1, :], -0.25)

    big2 = ctx.enter_context(tc.tile_pool(name="big2", bufs=1))

    # =====================================================================
    # transformer helpers
    # =====================================================================
    def rmsnorm_to_ag(w_row_ap, tgt_dot=False):
        ag_in = dram.tile([TLOC, H], BF16, tag="ag_in")
        w_exp = expand_row(act, "rowexp", w_row_ap, dt=BF16)
        tl = []
        for ti, (mo, ml) in enumerate(MT_LOC):
            xt = x_s[ti]
            fs = scr1.tile([ml, H], F32, tag="fscr")
            ssq_ = scr.tile([ml, 1], F32, tag=f"rms_s{ti}")
            nc.scalar.activation(fs[:], xt[:], AF.Square, accum_out=ssq_[:])
            sd_ = scr.tile([ml, 1], F32, tag=f"rms_d{ti}")
            nc.scalar.activation(sd_[:], ssq_[:], AF.Sqrt, scale=1.0 / H, bias=eps_sb[0:ml, :])
            rs_ = scr.tile([ml, 1], F32, tag=f"rms_r{ti}")
            nc.vector.reciprocal(rs_[:], sd_[:])
            xnb = act.tile([ml, H], BF16, tag="xnb")
            nc.scalar.activation(xnb[:], xt[:], AF.Copy, scale=rs_[:])
            nc.vector.tensor_mul(xnb[:], xnb[:], w_exp[0:ml, :])
            nc.sync.dma_start(ag_in[mo:mo + ml, :], xnb[:])
            if tgt_dot:
                tg = big2.tile([ml, H], F32, tag="fscr2")
                nc.sync.dma_start(tg[:], D["tgt_rows"][mo:mo + ml, :])
                pr = big2.tile([ml, H], F32, tag="fscr3")
                nc.vector.tensor_mul(pr[:], xnb[:], tg[:])
                t = scr.tile([ml, 1], F32, tag=f"tl{ti}")
                nc.vector.tensor_reduce(out=t[:], in_=pr[:], op=ADD, axis=AX)
                tl.append(t)
        return ag_in, tl

    def allgather_xT(ag_in):
        ag_out = dram.tile([NCORE, TLOC, H], BF16, tag="ag_out")
        nc.gpsimd.collective_compute("AllGather", mybir.AluOpType.bypass,
                                     replica_groups=[CORES],
                                     ins=[ag_in.opt()], outs=[ag_out.opt()])
        flat = ag_out[:].rearrange("c t h -> (c t) h")
        xT = []
        for k in range(H // 128):
            t = big2.tile([128, TOK], BF16, tag=f"xT{k}")
            nc.sync.dma_start_transpose(t[:], flat[:, k * 128:(k + 1) * 128])
            xT.append(t)
        return xT

    def dequant_strip(v_ap, s_ap, mo, ml, tagp):
        K = v_ap.shape[0]
        tiles = []
        kchunks = [(i * 128, min(128, K - i * 128)) for i in range((K + 127) // 128)]
        for ki, (ko, kl) in enumerate(kchunks):
            vt = wstream.tile([kl, ml], BF16, tag=f"wv_{tagp}")
            nc.sync.dma_start(vt[:], v_ap[ko:ko + kl, mo:mo + ml])
            st = wstream.tile([kl, ml], F32, tag=f"ws_{tagp}")
            for bi in range(kl // BLK):
                brow = ko // BLK + bi
                nc.sync.dma_start(st[bi * BLK:(bi + 1) * BLK, :],
                                  s_ap[brow:brow + 1, mo:mo + ml].to_broadcast((BLK, ml)))
            wt = wkeep.tile([kl, ml], BF16, tag=f"wd_{tagp}{ki}")
            nc.vector.tensor_mul(wt[:], vt[:], st[:])
            tiles.append(wt)
        return tiles

    def lora_mid(a_ap, xT_tiles, nk):
        aT = []
        for k in range(nk):
            t = wkeep.tile([128, R], BF16, tag=f"aT{k}")
            nc.sync.dma_start(t[:], a_ap[k * 128:(k + 1) * 128, :])
            aT.append(t)
        mid = act.tile([R, TOK], BF16, tag="mid")
        for (no, nl) in TOKCH:
            ps = psA.tile([R, 512], F32, tag="ps_main")
            for k in range(nk):
                nc.tensor.matmul(ps[:, 0:nl], aT[k][:], xT_tiles[k][:, no:no + nl],
                                 start=(k == 0), stop=(k == nk - 1))
            nc.scalar.activation(mid[:, no:no + nl], ps[:, 0:nl], AF.Copy, scale=LORA_SCALE)
        return mid

    def rope_evict(ps, dst, no, nl):
        po = no % SB
        c_sl = cos_sb[:, po:po + nl]
        s_sl = sin_sb[:, po:po + nl]
        t1 = scr.tile([64, 512], F32, tag="rope1")
        t2 = scr.tile([64, 512], F32, tag="rope2")
        nc.vector.tensor_mul(t1[:, 0:nl], ps[0:64, 0:nl], c_sl)
        nc.vector.tensor_mul(t2[:, 0:nl], ps[64:128, 0:nl], s_sl)
        nc.vector.tensor_sub(dst[0:64, no:no + nl], t1[:, 0:nl], t2[:, 0:nl])
        nc.vector.tensor_mul(t1[:, 0:nl], ps[0:64, 0:nl], s_sl)
        nc.vector.tensor_mul(t2[:, 0:nl], ps[64:128, 0:nl], c_sl)
        nc.vector.tensor_add(dst[64:128, no:no + nl], t1[:, 0:nl], t2[:, 0:nl])

    def rs_and_add(rs_in):
        rs_out = dram.tile([TLOC, H], BF16, tag="rs_out")
        nc.gpsimd.collective_compute("ReduceScatter", ADD, replica_groups=[CORES],
                                     ins=[rs_in.opt()], outs=[rs_out.opt()])
        for ti, (mo, ml) in enumerate(MT_LOC):
            rt = act.tile([ml, H], BF16, tag="xnb")
            nc.sync.dma_start(rt[:], rs_out[mo:mo + ml, :])
            nc.vector.tensor_add(x_s[ti][:], x_s[ti][:], rt[:])

    def tokm(i):
        t = big2.tile([128, TOK], BF16, tag=f"tokm{i}")
        return t

    # =====================================================================
    # transformer layers
    # =====================================================================
    NKH = H // 128
    for l in range(L):
        ag_in, _ = rmsnorm_to_ag(D["attn_w"][l, 0:1, :])
        xT = allgather_xT(ag_in)

        qkv_sb = {}
        for pi, (name, v_ap, s_ap, a_ap, b_ap, do_rope) in enumerate((
            ("q", D["wq_v"][l], D["wq_s"][l], D["aq"][l], D["bq"][l], True),
            ("k", D["wk_v"][l], D["wk_s"][l], D["ak"][l], D["bk"][l], True),
            ("v", D["wv_v"][l], D["wv_s"][l], D["av"][l], D["bv"][l], False),
        )):
            mid = lora_mid(a_ap, xT, NKH)
            dst = []
            for m in range(HLOC):
                dtile = tokm(pi * 2 + m)
                dst.append(dtile)
            for m in range(HLOC):
                bT = wstream.tile([R, 128], BF16, tag="bT")
                nc.sync.dma_start(bT[:], b_ap[:, m * 128:(m + 1) * 128])
                wts = dequant_strip(v_ap, s_ap, m * 128, 128, "qkv")
                for (no, nl) in TOKCH:
                    ps = psA.tile([128, 512], F32, tag="ps_main")
                    for k in range(NKH):
                        nc.tensor.matmul(ps[:, 0:nl], wts[k][:], xT[k][:, no:no + nl],
                                         start=(k == 0), stop=False)
                    nc.tensor.matmul(ps[:, 0:nl], bT[:], mid[:, no:no + nl],
                                     start=False, stop=True)
                    if do_rope:
                        rope_evict(ps, dst[m], no, nl)
                    else:
                        nc.vector.tensor_copy(dst[m][:, no:no + nl], ps[:, 0:nl])
            qkv_sb[name] = dst

        o_sb = []
        for m in range(HLOC):
            otile = tokm(6 + m)
            o_sb.append(otile)
        for m in range(HLOC):
            qh, kh, vh = qkv_sb["q"][m], qkv_sb["k"][m], qkv_sb["v"][m]
            for b in range(B):
                boff = b * SB
                vT = []
                for ki, (ko, kl) in enumerate(QT_ATT):
                    pst = psB.tile([128, 128], BF16, tag="ps_small")
                    nc.tensor.transpose(pst[0:kl, :], vh[:, boff + ko:boff + ko + kl],
                                        ident[:, :])
                    t = scr.tile([128, 128], BF16, tag=f"vT{ki}")
                    nc.vector.tensor_copy(t[0:kl, :], pst[0:kl, :])
                    vT.append(t)
                for qi, (qo, ql) in enumerate(QT_ATT):
                    sb_ = scr.tile([ql, SB], F32, tag="sb_score")
                    for (no2, nl2) in ((0, 512), (512, 32)):
                        ps_s = psA.tile([ql, 512], F32, tag="ps_main")
                        nc.tensor.matmul(ps_s[:, 0:nl2],
                                         qh[:, boff + qo:boff + qo + ql],
                                         kh[:, boff + no2:boff + no2 + nl2],
                                         start=True, stop=True)
                        nc.vector.scalar_tensor_tensor(sb_[:, no2:no2 + nl2], ps_s[:, 0:nl2],
                                                       INV_SQRT_HD,
                                                       bias_sb[(b, qi)][:, no2:no2 + nl2],
                                                       MULT, ADD)
                    mrow = scr.tile([ql, 1], F32, tag="mrow")
                    nc.vector.tensor_reduce(out=mrow[:], in_=sb_[:], op=MAXOP, axis=AX)
                    nmrow = scr.tile([ql, 1], F32, tag="nmrow")
                    nc.vector.tensor_scalar_mul(nmrow[:], mrow[:], -1.0)
                    pbf = scr.tile([ql, SB], BF16, tag="pbf")
                    den = scr.tile([ql, 1], F32, tag="den")
                    nc.scalar.activation(pbf[:], sb_[:], AF.Exp, bias=nmrow[:], accum_out=den[:])
                    rden = scr.tile([ql, 1], F32, tag="rden")
                    nc.vector.reciprocal(rden[:], den[:])
                    pn = scr.tile([ql, SB], BF16, tag="pn")
                    nc.scalar.activation(pn[:], pbf[:], AF.Copy, scale=rden[:])
                    ps_o = psO.tile([128, 128], F32, tag="ps_o")
                    for ki, (ko, kl) in enumerate(QT_ATT):
                        pst = psB.tile([128, 128], BF16, tag="ps_small")
                        nc.tensor.transpose(pst[0:kl, 0:ql], pn[:, ko:ko + kl],
                                            ident[0:ql, 0:ql])
                        pT = scr.tile([128, 128], BF16, tag="pT")
                        nc.vector.tensor_copy(pT[0:kl, 0:ql], pst[0:kl, 0:ql])
                        nc.tensor.matmul(ps_o[:, 0:ql], vT[ki][0:kl, :], pT[0:kl, 0:ql],
                                         start=(ki == 0), stop=(ki == len(QT_ATT) - 1))
                    nc.vector.tensor_copy(o_sb[m][:, boff + qo:boff + qo + ql], ps_o[:, 0:ql])

        # o projection + lora-o
        aoT = []
        for k in range(HLOC):
            aot = wkeep.tile([128, R], BF16, tag=f"aT{k}")
            nc.sync.dma_start(aot[:], D["ao"][l, k * 128:(k + 1) * 128, :])
            aoT.append(aot)
        mid_o = act.tile([R, TOK], BF16, tag="mid")
        for (no, nl) in TOKCH:
            ps = psA.tile([R, 512], F32, tag="ps_main")
            for k in range(HLOC):
                nc.tensor.matmul(ps[:, 0:nl], aoT[k][:], o_sb[k][:, no:no + nl],
                                 start=(k == 0), stop=(k == HLOC - 1))
            nc.scalar.activation(mid_o[:, no:no + nl], ps[:, 0:nl], AF.Copy, scale=LORA_SCALE)
        bo_sb = wkeep.tile([R, H], BF16, tag="bo_sb")
        nc.sync.dma_start(bo_sb[:], D["bo"][l])
        rs_in = dram.tile([TOK, H], BF16, tag="rs_in")
        for n in range(H // 512):
            wts = dequant_strip(D["wo_v"][l], D["wo_s"][l], n * 512, 512, "od")
            for (mo, ml) in MT_TOK:
                ps = psA.tile([ml, 512], F32, tag="ps_main")
                for k in range(HLOC):
                    nc.tensor.matmul(ps[:], o_sb[k][:, mo:mo + ml], wts[k][:],
                                     start=(k == 0), stop=False)
                nc.tensor.matmul(ps[:], mid_o[:, mo:mo + ml], bo_sb[:, n * 512:(n + 1) * 512],
                                 start=False, stop=True)
                ob = scr.tile([ml, 512], BF16, tag="o_ev")
                nc.vector.tensor_copy(ob[:], ps[:])
                nc.sync.dma_start(rs_in[mo:mo + ml, n * 512:(n + 1) * 512], ob[:])
        rs_and_add(rs_in)
        if dbg and l == 0:
            nc.sync.dma_start(dbg["xs1"][0:128, :], x_s[0][:])
            nc.sync.dma_start(dbg["xs1"][128:TLOC, :], x_s[1][:])

        # MLP
        ag_in2, _ = rmsnorm_to_ag(D["mlp_w"][l, 0:1, :])
        xT2 = allgather_xT(ag_in2)
        ffa = []
        for mi, (mo, ml) in enumerate(MT_FFN):
            ftile = tokm(mi)
            ffa.append(ftile)
        for mi, (mo, ml) in enumerate(MT_FFN):
            wts_g = dequant_strip(D["wg_v"][l], D["wg_s"][l], mo, ml, "g")
            wts_u = dequant_strip(D["wu_v"][l], D["wu_s"][l], mo, ml, "u")
            for (no, nl) in TOKCH:
                ps_g = psA.tile([ml, 512], F32, tag="ps_main")
                for k in range(NKH):
                    nc.tensor.matmul(ps_g[:, 0:nl], wts_g[k][:], xT2[k][:, no:no + nl],
                                     start=(k == 0), stop=(k == NKH - 1))
                gact = scr.tile([ml, 512], BF16, tag="gact")
                nc.scalar.activation(gact[:, 0:nl], ps_g[:, 0:nl], AF.Silu)
                ps_u = psA.tile([ml, 512], F32, tag="ps_main")
                for k in range(NKH):
                    nc.tensor.matmul(ps_u[:, 0:nl], wts_u[k][:], xT2[k][:, no:no + nl],
                                     start=(k == 0), stop=(k == NKH - 1))
                nc.vector.tensor_mul(ffa[mi][0:ml, no:no + nl], gact[:, 0:nl], ps_u[:, 0:nl])
        rs_in2 = dram.tile([TOK, H], BF16, tag="rs_in")
        for n in range(H // 512):
            wts_d = dequant_strip(D["wd_v"][l], D["wd_s"][l], n * 512, 512, "od")
            for (mo, ml) in MT_TOK:
                ps = psA.tile([ml, 512], F32, tag="ps_main")
                for ki, (fo, fl) in enumerate(MT_FFN):
                    nc.tensor.matmul(ps[:], ffa[ki][0:fl, mo:mo + ml], wts_d[ki][:],
                                     start=(ki == 0), stop=(ki == len(MT_FFN) - 1))
                ob = scr.tile([ml, 512], BF16, tag="o_ev")
                nc.vector.tensor_copy(ob[:], ps[:])
                nc.sync.dma_start(rs_in2[mo:mo + ml, n * 512:(n + 1) * 512], ob[:])
        rs_and_add(rs_in2)

    if dbg:
        nc.sync.dma_start(dbg["xs2"][0:128, :], x_s[0][:])
        nc.sync.dma_start(dbg["xs2"][128:TLOC, :], x_s[1][:])

    # =====================================================================
    # final norm, lm_head CE, loss
    # =====================================================================
    ag_in3, tl = rmsnorm_to_ag(D["fin_w"][0:1, :], tgt_dot=True)
    xT3 = allgather_xT(ag_in3)

    NV = VLOC // 500  # 8
    se_in = dram1.tile([TOK, 1], F32)
    sums_t = {}
    for (mo, ml) in MT_TOK:
        stile = act.tile([ml, NV], F32, tag=f"sums{mo}")
        sums_t[mo] = stile
    for grp in (MT_TOK[0:3], MT_TOK[3:6], MT_TOK[6:9]):
        for nv in range(NV):
            pss = {}
            for gi, (mo, ml) in enumerate(grp):
                lmtile = psLM.tile([ml, 500], F32, tag=f"ps_lm{gi}")
                pss[mo] = lmtile
            for k in range(NKH):
                lw = wstream.tile([128, 500], BF16, tag="lmh_w")
                nc.sync.dma_start(lw[:], D["lmh"][k * 128:(k + 1) * 128, nv * 500:(nv + 1) * 500])
                for (mo, ml) in grp:
                    nc.tensor.matmul(pss[mo][:], xT3[k][:, mo:mo + ml], lw[:],
                                     start=(k == 0), stop=(k == NKH - 1))
            for (mo, ml) in grp:
                esc = scr.tile([ml, 500], BF16, tag="e_scr")
                nc.scalar.activation(esc[:], pss[mo][:], AF.Exp,
                                     accum_out=sums_t[mo][:, nv:nv + 1])
    for (mo, ml) in MT_TOK:
        se = scr.tile([ml, 1], F32, tag="se")
        nc.vector.tensor_reduce(out=se[:], in_=sums_t[mo][:], op=ADD, axis=AX)
        nc.sync.dma_start(se_in[mo:mo + ml, :], se[:])
    se_out = dram1.tile([TLOC, 1], F32)
    nc.gpsimd.collective_compute("ReduceScatter", ADD, replica_groups=[CORES],
                                 ins=[se_in.opt()], outs=[se_out.opt()])
    parts = []
    for ti, (mo, ml) in enumerate(MT_LOC):
        seg = scr.tile([ml, 1], F32, tag=f"seg{ti}")
        nc.sync.dma_start(seg[:], se_out[mo:mo + ml, :])
        lse = scr.tile([ml, 1], F32, tag=f"lseg{ti}")
        nc.scalar.activation(lse[:], seg[:], AF.Ln)
        nc.vector.tensor_sub(lse[:], lse[:], tl[ti][:])
        wv = scr.tile([ml, 1], F32, tag=f"wvt{ti}")
        nc.sync.dma_start(wv[:], D["nll_w"][mo:mo + ml, :])
        nc.vector.tensor_mul(lse[:], lse[:], wv[:])
        red = scr.tile([ml, 1], F32, tag=f"red{ti}")
        nc.gpsimd.partition_all_reduce(red[:], lse[:], channels=ml,
                                       reduce_op=bass.bass_isa.ReduceOp.add)
        parts.append(red)
    nll_part = scr.tile([1, 1], F32, tag="nllp")
    nc.vector.tensor_add(nll_part[:], parts[0][0:1, :], parts[1][0:1, :])
    sb8 = scr.tile([8, 1], F32, tag="sb8")
    nc.any.memset(sb8[:], 0.0)
    nc.vector.tensor_copy(sb8[0:1, :], nll_part[:])
    cc_nll_in = dram1.tile([8, 1], F32)
    cc_nll_out = dram1.tile([8, 1], F32)
    nc.sync.dma_start(cc_nll_in[:], sb8[:])
    nc.gpsimd.collective_compute("AllReduce", ADD, replica_groups=[CORES],
                                 ins=[cc_nll_in.opt()], outs=[cc_nll_out.opt()])
    lm_loss = scr.tile([1, 1], F32, tag="lml")
    nc.sync.dma_start(lm_loss[:], cc_nll_out[0:1, :])
    half_c = scr.tile([1, 1], F32, tag="halfc")
    nc.vector.tensor_scalar_mul(half_c[:], contrast[:], 0.5)
    total = scr.tile([1, 1], F32, tag="total")
    nc.vector.tensor_add(total[:], lm_loss[:], half_c[:])
    nc.sync.dma_start(out_loss[:], total[:])
    if dbg:
        nc.sync.dma_start(dbg["scal"][0:1, :], contrast[:])
        nc.sync.dma_start(dbg["scal"][1:2, :], lm_loss[:])
        nc.sync.dma_start(dbg["scal"][2:3, :], nll_part[:])
        nc.sync.dma_start(dbg["scal"][3:8, :], sb8[0:5, :])


# ---------------------------------------------------------------------------
# host-side prep + entry point
# ---------------------------------------------------------------------------

def _prep_inputs(input_ids, attention_mask, labels, visual_features, acoustic_features,
                 params, qcodes):
    """Build the 8 per-core input maps (all numpy)."""
    embed = _f32(params["embed"])
    lm_head = _f32(params["lm_head"])
    in_maps = [dict() for _ in range(NCORE)]

    def put(name, fn):
        for c in range(NCORE):
            in_maps[c][name] = fn(c)

    # ---- projector group assignment: cores 0-3 visual, 4-7 acoustic ----
    vp, ap_ = params["vp"], params["ap"]
    vis = _f32(visual_features).reshape(B * VT, VD)       # 512 x 1024
    aco = _f32(acoustic_features).reshape(B * AT, AD)     # 1024 x 768

    def proj_inputs(c):
        if c < 4:
            feats, d, ntok_core = vis, VD, (B * VT) // 4      # 128
        else:
            feats, d, ntok_core = aco, AD, (B * AT) // 4      # 256
        i = c % 4
        sl = feats[i * ntok_core:(i + 1) * ntok_core]
        x = np.zeros((PT, PD), np.float32)
        x[:ntok_core, :d] = sl
        return _bf(x.T)                                        # [PD, PT]

    put("pt_inT", proj_inputs)

    def proj_w(c, key, shape_pad):
        p = vp if c < 4 else ap_
        w = _f32(p[key])
        out = np.zeros(shape_pad, np.float32)
        out[:w.shape[0], :w.shape[1]] = w
        return out

    put("pw1", lambda c: _bf(proj_w(c, "w1", (P2D, PD)).T))
    put("pw2", lambda c: _bf(proj_w(c, "w2", (H, P2D)).T))

    def proj_b1(c):
        p = vp if c < 4 else ap_
        b1 = np.zeros(P2D, np.float32)
        b1[:p["b1"].shape[0]] = _f32(p["b1"])
        return _f32(b1.reshape(P2D // 128, 128).T)            # [128, 16]

    put("pb1", proj_b1)
    put("pb2", lambda c: _f32((vp if c < 4 else ap_)["b2"]).reshape(1, H))
    put("png", lambda c: _f32((vp if c < 4 else ap_)["g"]).reshape(1, H))
    put("pnb", lambda c: _f32((vp if c < 4 else ap_)["b"]).reshape(1, H))

    def pmean_sel(c):
        # [PT, 4] column j weight: j = modality*? ordering: [ve_b0, ae_b0, ve_b1, ae_b1]
        selw = np.zeros((PT, 4), np.float32)
        if c < 4:
            ntok_core, seqlen, col0 = (B * VT) // 4, VT, 0
        else:
            ntok_core, seqlen, col0 = (B * AT) // 4, AT, 1
        i = c % 4
        for t in range(ntok_core):
            gidx = i * ntok_core + t           # global flattened token
            b = gidx // seqlen
            selw[t, col0 + 2 * b] = 1.0 / seqlen
        return _bf(selw)

    put("pmean_sel", pmean_sel)

    # ---- token embeddings / residual init ----
    ids = np.asarray(input_ids)
    te_full = np.zeros((TOK, H), np.float32)
    for b in range(B):
        te_full[b * SB + 2: b * SB + 2 + T] = embed[ids[b]]
    put("te_loc", lambda c: _f32(te_full[c * TLOC:(c + 1) * TLOC]))

    def emb_selT(c):
        sel = np.zeros((4, TLOC), np.float32)
        for t in range(TLOC):
            g = c * TLOC + t
            b, s_ = g // SB, g % SB
            if s_ == 0:
                sel[2 * b + 0, t] = 1.0
            elif s_ == 1:
                sel[2 * b + 1, t] = 1.0
        return _bf(sel)

    put("emb_selT", emb_selT)

    # ---- norms ----
    attn_w = np.stack([_f32(params["layers"][l]["attn_norm"]) for l in range(L)])[:, None, :]
    mlp_w = np.stack([_f32(params["layers"][l]["mlp_norm"]) for l in range(L)])[:, None, :]
    put("attn_w", lambda c: _f32(attn_w))
    put("mlp_w", lambda c: _f32(mlp_w))
    put("fin_w", lambda c: _f32(params["final_norm"]).reshape(1, H))

    # ---- rope tables ----
    half = HD // 2
    freqs = 1.0 / (10000.0 ** (np.arange(half, dtype=np.float32) / half))
    ang = np.arange(SB, dtype=np.float32)[:, None] * freqs[None, :]   # [SB, 64]
    put("cos_t", lambda c: _f32(np.cos(ang).T))
    put("sin_t", lambda c: _f32(np.sin(ang).T))

    # ---- attention bias [B, SB, SB] ----
    mask = _f32(attention_mask)
    bias = np.full((B, SB, SB), -1e9, np.float32)
    for b in range(B):
        am = np.ones(SB, np.float32) * 0.0
        am[:2] = 1.0
        am[2:2 + T] = mask[b]
        allowed = np.tril(np.ones((SB, SB), np.float32)) * am[None, :]
        allowed[:, S:] = 0.0
        allowed[S:, :] = 1.0   # pad queries attend everything (harmless, avoids NaN)
        bias[b] = np.where(allowed > 0, 0.0, -1e9)
    put("bias", lambda c: _bf(bias))

    # ---- per-layer quantized weights ----
    def qw(l, name):
        codes = np.asarray(qcodes["layers"][l][name])
        pf = params["layers"][l][name]
        vals = NF4[codes]                      # [od, idim] f32
        scales = _f32(pf["scales"])            # [od, idim//BLK]
        return vals, scales, pf

    def stack_l(fn):
        return np.stack([fn(l) for l in range(L)])

    for nm, key in (("q", "wq"), ("k", "wk"), ("v", "wv")):
        def mk(c, nm=nm):
            vs, ss = [], []
            for l in range(L):
                vals, scales, _ = qw(l, nm)
                sl = slice(c * OQ, (c + 1) * OQ)
                vs.append(vals[sl].T)                 # [H, OQ]
                ss.append(scales[sl].T)               # [H//BLK, OQ]
            return np.stack(vs), np.stack(ss)
        put(f"{key}_v", lambda c, mk=mk: _bf(mk(c)[0]))
        put(f"{key}_s", lambda c, mk=mk: _f32(mk(c)[1]))

    def mk_o(c):
        vs, ss = [], []
        for l in range(L):
            vals, scales, _ = qw(l, "o")
            # o: y = x @ Wo.T, shard over input dim (columns of Wo) = my heads
            sl = slice(c * OQ, (c + 1) * OQ)
            vs.append(vals[:, sl].T)                  # [OQ, H]
            # scales indexed by input-block: blocks [c*OQ/BLK, ...)
            bsl = slice(c * OQ // BLK, (c + 1) * OQ // BLK)
            ss.append(scales[:, bsl].T)               # [OQ//BLK, H]
        return np.stack(vs), np.stack(ss)

    put("wo_v", lambda c: _bf(mk_o(c)[0]))
    put("wo_s", lambda c: _f32(mk_o(c)[1]))

    for nm, key in (("gate", "wg"), ("up", "wu")):
        def mkg(c, nm=nm):
            vs, ss = [], []
            for l in range(L):
                vals, scales, _ = qw(l, nm)
                sl = slice(c * FLOC, (c + 1) * FLOC)
                vs.append(vals[sl].T)                 # [H, FLOC]
                ss.append(scales[sl].T)
            return np.stack(vs), np.stack(ss)
        put(f"{key}_v", lambda c, mkg=mkg: _bf(mkg(c)[0]))
        put(f"{key}_s", lambda c, mkg=mkg: _f32(mkg(c)[1]))

    def mk_d(c):
        vs, ss = [], []
        for l in range(L):
            vals, scales, _ = qw(l, "down")
            sl = slice(c * FLOC, (c + 1) * FLOC)
            vs.append(vals[:, sl].T)                  # [FLOC, H]
            bsl = slice(c * FLOC // BLK, (c + 1) * FLOC // BLK)
            ss.append(scales[:, bsl].T)               # [FLOC//BLK, H]
        return np.stack(vs), np.stack(ss)

    put("wd_v", lambda c: _bf(mk_d(c)[0]))
    put("wd_s", lambda c: _f32(mk_d(c)[1]))

    # ---- lora ----
    for nm, akey, bkey in (("q", "aq", "bq"), ("k", "ak", "bk"), ("v", "av", "bv")):
        def mka(c, nm=nm):
            return np.stack([_f32(params["layers"][l][nm]["A"]).T for l in range(L)])
        def mkb(c, nm=nm):
            return np.stack([_f32(params["layers"][l][nm]["B"])[c * OQ:(c + 1) * OQ].T
                             for l in range(L)])
        put(akey, lambda c, mka=mka: _bf(mka(c)))
        put(bkey, lambda c, mkb=mkb: _bf(mkb(c)))

    def mk_ao(c):
        return np.stack([_f32(params["layers"][l]["o"]["A"])[:, c * OQ:(c + 1) * OQ].T
                         for l in range(L)])          # [OQ, R]

    def mk_bo(c):
        return np.stack([_f32(params["layers"][l]["o"]["B"]).T for l in range(L)])  # [R, H]

    put("ao", lambda c: _bf(mk_ao(c)))
    put("bo", lambda c: _bf(mk_bo(c)))

    # ---- lm head / loss ----
    put("lmh", lambda c: _bf(lm_head[c * VLOC:(c + 1) * VLOC].T))

    labs = np.asarray(labels)
    tgt_full = np.zeros((TOK, H), np.float32)
    nw_full = np.zeros((TOK, 1), np.float32)
    n_valid = 0
    for b in range(B):
        for s_ in range(1, T + 1):        # in-batch positions with valid next-label
            tid = int(labs[b, s_ - 1])
            if tid >= 0:
                g = b * SB + s_
                tgt_full[g] = lm_head[tid]
                nw_full[g, 0] = 1.0
                n_valid += 1
    nw_full /= max(n_valid, 1)
    put("tgt_rows", lambda c: _f32(tgt_full[c * TLOC:(c + 1) * TLOC]))
    put("nll_w", lambda c: _f32(nw_full[c * TLOC:(c + 1) * TLOC]))

    tc_rows = embed[ids[:, 0]]                         # [2, H]
    put("tc_rows", lambda c: _f32(tc_rows))
    put("tc_kxm", lambda c: _bf(tc_rows.T))
    put("dsel", lambda c: _f32(np.array([[1, 0], [1, 0], [0, 1], [0, 1]], np.float32)))
    put("i2", lambda c: _bf(np.eye(2, dtype=np.float32)))
    put("temp", lambda c: _f32(np.array([[float(params["temp"])]])))
    return in_maps


def kernel(input_ids, attention_mask, labels, visual_features, acoustic_features,
           params, qcodes, _debug=False, _trace=False):
    key = ("nc", bool(_debug))
    if key not in _CACHE:
        _CACHE[key] = build_nc(debug_outputs=_debug)
    nc = _CACHE[key]
    in_maps = _prep_inputs(input_ids, attention_mask, labels, visual_features,
                           acoustic_features, params, qcodes)
    res = run_bass_kernel_spmd(nc, in_maps, list(range(NCORE)), trace=_trace)
    out = np.float32(res.results[0]["loss"][0, 0])
    if _debug or _trace:
        kernel.last_results = res
    return np.asarray(out, dtype=np.float32)
